# revision 15
# baseline (speedup 1.0000x reference)
"""Multi-head self-attention (B=2, S=2048, D=1024, H=16) on 8 Trainium2 cores.

Sharding: Megatron-style tensor parallelism on the head dimension.
Each core owns 2 heads (128 of the 1024 model dims):
  - Wq/Wk/Wv column-sharded: core c computes Q/K/V for dims [c*128,(c+1)*128)
  - attention for its 2 heads over both batches
  - Wo row-sharded: core c produces a partial output [4096, 1024] (bf16)
  - host sums the 8 partials and adds bo.

All matmuls are bf16 (fp32 accumulate). fp8 variants were measured and
rejected: any noise sigma on the softmax logits appears ~1:1 as relative
error on the output (the context is a weighted mean whose magnitude
shrinks by the same sqrt(N) as the noise), so fp8 Q/K (5% logit noise)
blows the 2e-2 error budget.

Performance structure (trace-driven; PE issues a 512-free bf16 matmul
every ~216ns when dense, and idle gaps also drop the HAM clock 2.4->1.2
GHz, so everything aims at PE density):
  - phases alternate per batch (proj b0 -> attention b0 -> proj b1 ->
    attention b1) so the projection/attention boundary stall is paid
    once, hidden inside the attention stretch.
  - phase B is software-pipelined: the PV matmuls of key-tile kt-PV_LAG
    are emitted between the score matmuls of kt.
  - exp: 12 of 16 key tiles per q-chunk on the scalar engine (true exp);
    4 on vector+pool via a two-phase Schraudolph: two int16
    bits-of-bf16 approximations S1, S2 (rne(A*s+B), the DVE f32->int16
    convert is RNE on HW) whose half-period-offset sawtooths cancel in
    S1 + 1.1653*S2 down to +-1.35%. On 4/16 tiles that adds ~7e-3
    output error in quadrature. The combine runs on the Pool engine
    (SBUF-only reads; only ACT/DVE can read PSUM, which is the real
    shared resource the split relieves).
  - PSUM->SBUF output casts split 2:6 between scalar and vector.
  - the ones-column folded into V makes the PV matmul also produce the
    softmax denominator; no max subtraction needed (scores*0.125 are
    ~N(0,1) for this problem family).
  - x is DMA'd in 8 big [128,4,1024] transfers (dma_start issue time is
    ~0.6us each on the queueing engine).
  - output partials are written bf16; the host accumulates in f64.
"""

import numpy as np
import ml_dtypes
from contextlib import ExitStack

import concourse.bass as bass
import concourse.tile as tile
from concourse import bacc, mybir
from concourse.bass_utils import run_bass_kernel_spmd
from concourse.masks import make_identity

B, S, D = 2, 2048, 1024
H, DH = 16, 64
T = B * S                  # 4096 tokens total
N_CORES = 8
OPC = D // N_CORES         # 128 out dims per core
HPC = H // N_CORES         # 2 heads per core
NI = D // 128              # 8 contraction chunks of 128
TCH = 512                  # projection token chunk
QCH = 512                  # attention q chunk
NQCH = S // QCH            # 4 per batch
NKT = S // 128             # 16 key tiles per batch
HW = DH + 2                # 66 cols per head in the v tile (data|ones|pad)
VW = HPC * HW              # 132

F32 = mybir.dt.float32
BF16 = mybir.dt.bfloat16
I16 = mybir.dt.int16
EXP = mybir.ActivationFunctionType.Exp
MULT = mybir.AluOpType.mult
ADD = mybir.AluOpType.add

MM_DT, MM_NP = BF16, ml_dtypes.bfloat16

PV_LAG = 2

# two-phase Schraudolph constants: at = S(B1) + 1.16532*S(B2) where
# S(b) = bitcast_bf16(rne(A*s + 16256 + b)) ~ exp(0.125*s); max rel
# error +-1.35% (vs +-4.1% single-phase)
A_SCH = 128.0 * 1.4426950408889634 * 0.125
B_SCH1 = 127.0 * 128.0 - 185.2977
B_SCH2 = 127.0 * 128.0 - 122.7560
C_SCH2 = 1.1653249
DVE_KT = frozenset((5, 10, 15))


def _mha_kernel(tc, y, xT16, wq, wk, wv, woT, bq, bk, bv):
    with ExitStack() as ctx:
        _mha_kernel_inner(ctx, tc, y, xT16, wq, wk, wv, woT, bq, bk, bv)


def _mha_kernel_inner(ctx, tc, y, xT16, wq, wk, wv, woT, bq, bk, bv):
    nc = tc.nc
    pers = ctx.enter_context(tc.tile_pool(name="pers", bufs=1))

    qT = pers.tile([128, T], MM_DT, tag="qT")
    kT = pers.tile([128, T], MM_DT, tag="kT")
    vT = pers.tile([128, T], MM_DT, tag="vT")
    vtk = pers.tile([128, B * NKT, VW], MM_DT, tag="vtk")
    wq_sb = pers.tile([128, NI, OPC], MM_DT, tag="wq")
    wk_sb = pers.tile([128, NI, OPC], MM_DT, tag="wk")
    wv_sb = pers.tile([128, NI, OPC], MM_DT, tag="wv")
    woT_sb = pers.tile([128, D], MM_DT, tag="wo")
    bq_sb = pers.tile([128, 1], F32, tag="bq")
    bk_sb = pers.tile([128, 1], F32, tag="bk")
    bv_sb = pers.tile([128, 1], F32, tag="bv")
    ident = pers.tile([128, 128], MM_DT, tag="ident")

    # identity first: it shares the gpsimd queue with the weight DMAs
    # and the chunk-0 V transposes wait on it. One DMA per weight tensor.
    make_identity(nc, ident)
    nc.gpsimd.dma_start(wq_sb[:, :, :], wq[:, :, :])
    nc.gpsimd.dma_start(wk_sb[:, :, :], wk[:, :, :])
    nc.gpsimd.dma_start(wv_sb[:, :, :], wv[:, :, :])
    nc.gpsimd.dma_start(woT_sb, woT)
    nc.gpsimd.dma_start(bq_sb, bq)
    nc.gpsimd.dma_start(bk_sb, bk)
    nc.gpsimd.dma_start(bv_sb, bv)
    # constant ones/pad columns of vtk
    onepad = pers.tile([128, 2], F32, tag="onepad")
    nc.vector.memset(onepad[:, 0:1], 1.0)
    nc.vector.memset(onepad[:, 1:2], 0.0)
    onepad_b = bass.AP(
        tensor=onepad.tensor,
        offset=onepad.offset,
        ap=[onepad.ap[0], [0, B * NKT], onepad.ap[1]],
    )
    for h in range(HPC):
        nc.vector.tensor_copy(
            vtk[:, :, h * HW + DH : h * HW + DH + 2], onepad_b
        )

    def proj_phase(ms):
        # Q/K/V projections for 1024-token double-chunks in `ms`; V is
        # transposed to token-major k-tiles right away.
        with (
            tc.tile_pool(name="psA", bufs=2, space="PSUM") as psA,
            tc.tile_pool(name="xin", bufs=4) as xin,
        ):
            for m in ms:
                xa = xin.tile([128, 4, 1024], MM_DT, tag="xt")
                nc.sync.dma_start(xa, xT16[0, :, :, m * 1024 : (m + 1) * 1024])
                xb = xin.tile([128, 4, 1024], MM_DT, tag="xt")
                nc.sync.dma_start(xb, xT16[1, :, :, m * 1024 : (m + 1) * 1024])
                for half in range(2):
                    t0 = m * 1024 + half * 512
                    hsl = slice(half * 512, (half + 1) * 512)
                    ps_q = psA.tile([128, TCH], F32, tag="ps_q")
                    ps_k = psA.tile([128, TCH], F32, tag="ps_k")
                    ps_v = psA.tile([128, TCH], F32, tag="ps_v")
                    for ps, w_sb in ((ps_q, wq_sb), (ps_k, wk_sb), (ps_v, wv_sb)):
                        for i in range(NI):
                            blk, u = divmod(i, 4)
                            xs = (xa if blk == 0 else xb)[:, u, hsl]
                            nc.tensor.matmul(
                                ps, w_sb[:, i, :], xs,
                                start=(i == 0), stop=(i == NI - 1),
                            )
                    sl = slice(t0, t0 + TCH)
                    nc.vector.tensor_scalar_add(qT[:, sl], ps_q, bq_sb)
                    nc.vector.tensor_scalar_add(kT[:, sl], ps_k, bk_sb)
                    nc.vector.tensor_scalar_add(vT[:, sl], ps_v, bv_sb)
                    for g in range(t0 // 128, (t0 + TCH) // 128):
                        ps_t = psA.tile([128, 128], MM_DT, tag="ps_t")
                        nc.tensor.transpose(
                            ps_t, vT[:, g * 128 : (g + 1) * 128], ident
                        )
                        for h in range(HPC):
                            nc.vector.tensor_copy(
                                vtk[:, g, h * HW : h * HW + DH],
                                ps_t[:, h * DH : (h + 1) * DH],
                            )

    def attn_phase(b):
        # attention + output projection for batch b, software-pipelined so
        # the PE stays dense (PV of kt-PV_LAG emitted between the score
        # matmuls of kt).
        with (
            tc.tile_pool(name="psB", bufs=2, space="PSUM") as psB,
            tc.tile_pool(name="att", bufs=8) as att,
            tc.tile_pool(name="sch", bufs=3) as schp,
            tc.tile_pool(name="sm", bufs=3) as sm,
            tc.tile_pool(name="yo_p", bufs=4) as yo_p,
        ):
            for qc in range(NQCH):
                q0 = b * S + qc * QCH
                ctx_sb = sm.tile([128, QCH], MM_DT, tag="ctx")
                at_tiles = []
                pvs = [
                    psB.tile([HW, QCH], F32, tag="ps_pv", name=f"pv{h}")
                    for h in range(HPC)
                ]

                def emit_pv(kt):
                    g = b * NKT + kt
                    for h in range(HPC):
                        nc.tensor.matmul(
                            pvs[h],
                            vtk[:, g, h * HW : (h + 1) * HW],
                            at_tiles[kt][:, h, :],
                            start=(kt == 0),
                            stop=(kt == NKT - 1),
                        )

                for kt in range(NKT):
                    g = b * NKT + kt
                    ps_s = psB.tile([128, 2, QCH], F32, tag="ps_s")
                    for h in range(HPC):
                        hs = slice(h * DH, (h + 1) * DH)
                        nc.tensor.matmul(
                            ps_s[:, h, :],
                            kT[hs, g * 128 : (g + 1) * 128],
                            qT[hs, q0 : q0 + QCH],
                            start=True,
                            stop=True,
                        )
                    at = att.tile([128, 2, QCH], MM_DT, tag="at")
                    if kt in DVE_KT:
                        # two-phase Schraudolph: DVE drains PSUM twice,
                        # Pool combines from SBUF
                        w1 = schp.tile([128, 2, QCH], I16, tag="w1")
                        w2 = schp.tile([128, 2, QCH], I16, tag="w2")
                        nc.vector.tensor_scalar(w1, ps_s, A_SCH, B_SCH1, MULT, ADD)
                        nc.vector.tensor_scalar(w2, ps_s, A_SCH, B_SCH2, MULT, ADD)
                        nc.vector.scalar_tensor_tensor(
                            at, w2.bitcast(BF16), C_SCH2, w1.bitcast(BF16),
                            MULT, ADD,
                        )
                    else:
                        nc.scalar.activation(at, ps_s, EXP, scale=0.125)
                    at_tiles.append(at)
                    if kt >= PV_LAG:
                        emit_pv(kt - PV_LAG)
                for kt in range(NKT - PV_LAG, NKT):
                    emit_pv(kt)

                for h in range(HPC):
                    # normalize: ctx rows for this head = pv[0:64] * recip(pv[64])
                    rraw = sm.tile([1, QCH], F32, tag="rraw")
                    nc.vector.tensor_copy(rraw, pvs[h][DH : DH + 1, :])
                    rrow = sm.tile([1, QCH], F32, tag="rrow")
                    nc.vector.reciprocal_approx_fast(rrow, rraw)
                    nrm = sm.tile([DH, QCH], F32, tag="nrm")
                    nc.gpsimd.partition_broadcast(nrm, rrow)
                    nc.vector.tensor_mul(
                        ctx_sb[h * DH : (h + 1) * DH, :], pvs[h][0:DH, :], nrm
                    )
                for t4 in range(QCH // 128):
                    yo = yo_p.tile([128, D], BF16, tag="yo")
                    for nch in range(D // 512):
                        ps_o = psB.tile([128, 512], F32, tag="ps_o")
                        nc.tensor.matmul(
                            ps_o,
                            ctx_sb[:, t4 * 128 : (t4 + 1) * 128],
                            woT_sb[:, nch * 512 : (nch + 1) * 512],
                            start=True,
                            stop=True,
                        )
                        dst = yo[:, nch * 512 : (nch + 1) * 512]
                        if t4 == 1 and nch == 1 or t4 == 3 and nch == 1:
                            nc.scalar.copy(dst, ps_o)
                        else:
                            nc.vector.tensor_copy(dst, ps_o)
                    r0 = q0 + t4 * 128
                    nc.sync.dma_start(y[r0 : r0 + 128, :], yo)

    proj_phase([0, 1])
    attn_phase(0)
    proj_phase([2, 3])
    attn_phase(1)


_NC_CACHE = {}


def _build_nc(repeats=1):
    if repeats in _NC_CACHE:
        return _NC_CACHE[repeats]
    nc = bacc.Bacc("TRN2", target_bir_lowering=False, debug=False, num_devices=N_CORES)
    xT16 = nc.dram_tensor("xT16", [2, 128, 4, T], MM_DT, kind="ExternalInput").ap()
    wq = nc.dram_tensor("wq", [128, NI, OPC], MM_DT, kind="ExternalInput").ap()
    wk = nc.dram_tensor("wk", [128, NI, OPC], MM_DT, kind="ExternalInput").ap()
    wv = nc.dram_tensor("wv", [128, NI, OPC], MM_DT, kind="ExternalInput").ap()
    woT = nc.dram_tensor("woT", [128, D], MM_DT, kind="ExternalInput").ap()
    bq = nc.dram_tensor("bq", [128, 1], F32, kind="ExternalInput").ap()
    bk = nc.dram_tensor("bk", [128, 1], F32, kind="ExternalInput").ap()
    bv = nc.dram_tensor("bv", [128, 1], F32, kind="ExternalInput").ap()
    y = nc.dram_tensor("y", [T, D], BF16, kind="ExternalOutput").ap()
    with tile.TileContext(nc) as tc:
        for _ in range(repeats):
            _mha_kernel(tc, y, xT16, wq, wk, wv, woT, bq, bk, bv)
    nc.compile()
    _NC_CACHE[repeats] = nc
    return nc


def _prep_in_maps(inputs):
    x = np.asarray(inputs["x"], np.float32)
    Wq = np.asarray(inputs["Wq"], np.float32)
    Wk = np.asarray(inputs["Wk"], np.float32)
    Wv = np.asarray(inputs["Wv"], np.float32)
    Wo = np.asarray(inputs["Wo"], np.float32)
    bq = np.asarray(inputs["bq"], np.float32)
    bk = np.asarray(inputs["bk"], np.float32)
    bv = np.asarray(inputs["bv"], np.float32)

    xTr = np.ascontiguousarray(x.reshape(T, D).T)  # [D, T]
    # xT16 [blk, p, u, t] = xTr[(blk*4+u)*128 + p, t]
    xT16_np = np.ascontiguousarray(
        xTr.reshape(2, 4, 128, T).transpose(0, 2, 1, 3)
    ).astype(MM_NP)

    def _w_slice(W, c):
        # [128(p), NI, OPC]: [p, i, o] = W[c*OPC+o, i*128+p]
        A = np.ascontiguousarray(W[c * OPC : (c + 1) * OPC, :].T)  # [D, OPC]
        return np.ascontiguousarray(A.reshape(NI, 128, OPC).transpose(1, 0, 2)).astype(
            MM_NP
        )

    in_maps = []
    for c in range(N_CORES):
        sl = slice(c * OPC, (c + 1) * OPC)
        in_maps.append(
            {
                "xT16": xT16_np,
                "wq": _w_slice(Wq, c),
                "wk": _w_slice(Wk, c),
                "wv": _w_slice(Wv, c),
                "woT": np.ascontiguousarray(Wo[:, sl].T).astype(MM_NP),
                "bq": bq[sl].reshape(OPC, 1).copy(),
                "bk": bk[sl].reshape(OPC, 1).copy(),
                "bv": bv[sl].reshape(OPC, 1).copy(),
            }
        )
    return in_maps


def kernel(**inputs) -> np.ndarray:
    nc = _build_nc()
    in_maps = _prep_in_maps(inputs)
    res = run_bass_kernel_spmd(nc, in_maps, core_ids=list(range(N_CORES)))
    bo = np.asarray(inputs["bo"], np.float32)
    y = np.zeros((T, D), np.float64)
    for c in range(N_CORES):
        y += res.results[c]["y"].astype(np.float64)
    y = (y + bo).astype(np.float32)
    return y.reshape(B, S, D)


# revision 17
# speedup vs baseline: 1.2087x; 1.2087x over previous
"""Multi-head self-attention (B=2, S=2048, D=1024, H=16) on 8 Trainium2 cores.

Sharding: Megatron-style tensor parallelism on the head dimension.
Each core owns 2 heads (128 of the 1024 model dims):
  - Wq/Wk/Wv column-sharded: core c computes Q/K/V for dims [c*128,(c+1)*128)
  - attention for its 2 heads over both batches
  - Wo row-sharded: core c produces a partial output [4096, 1024] (bf16)
  - host sums the 8 partials and adds bo.

All matmuls are bf16 (fp32 accumulate). fp8 variants were measured and
rejected: any noise sigma on the softmax logits appears ~1:1 as relative
error on the output (the context is a weighted mean whose magnitude
shrinks by the same sqrt(N) as the noise), so fp8 Q/K (5% logit noise)
blows the 2e-2 error budget.

Performance structure (trace-driven; a dense PE issues a 512-free bf16
matmul every ~216ns, and PE idle windows also drop the HAM clock
2.4->1.2 GHz, so everything aims at PE density):
  - phase B is software-pipelined two ways: the PV matmuls of key-tile
    kt-PV_LAG are emitted between the score matmuls of kt, and the
    normalize/output-projection tail of chunk c is emitted inside chunk
    c+1's score stretch, so the PE never drains at chunk boundaries
    (the 3.4us idle window that re-throttles the clock).
  - exp: 15 of 16 key tiles per q-chunk on the scalar engine (true
    exp); one on the vector engine via a two-phase Schraudolph: two
    int16 bits-of-bf16 approximations S1, S2 (bits = rne(A*s + B); the
    DVE f32->int16 convert is RNE on HW) whose half-period-offset
    sawtooths cancel to +-1.35% in S1 + 1.1653*S2. The combine rides
    the PV accumulation itself: S2 is multiplied against a pre-scaled
    copy of V (vtk_c = 1.1653*vtk, ones column included, so the
    softmax denominator combines identically). Only ACT/DVE can read
    PSUM, so the exp drain of the score matrix is the contended
    resource this splits.
  - the exp table set is preloaded by a dummy activation during phase A
    (first ACTIVATE otherwise pays ~2.7us of table DMA at the phase
    boundary).
  - the ones-column folded into V makes the PV matmul also produce the
    softmax denominator; no max subtraction needed (scores*0.125 are
    ~N(0,1) for this problem family).
  - x is DMA'd in 8 big [128,4,1024] transfers (dma_start issue time is
    ~0.6us each on the queueing engine).
  - output partials are written bf16; the host accumulates in f64.
"""

import numpy as np
import ml_dtypes
from contextlib import ExitStack

import concourse.bass as bass
import concourse.tile as tile
from concourse import bacc, mybir
from concourse.bass_utils import run_bass_kernel_spmd
from concourse.masks import make_identity

B, S, D = 2, 2048, 1024
H, DH = 16, 64
T = B * S                  # 4096 tokens total
N_CORES = 8
OPC = D // N_CORES         # 128 out dims per core
HPC = H // N_CORES         # 2 heads per core
NI = D // 128              # 8 contraction chunks of 128
TCH = 512                  # projection token chunk
QCH = 512                  # attention q chunk
NQCH = S // QCH            # 4 per batch
NKT = S // 128             # 16 key tiles per batch
HW = DH + 2                # 66 cols per head in the v tile (data|ones|pad)
VW = HPC * HW              # 132

F32 = mybir.dt.float32
BF16 = mybir.dt.bfloat16
I16 = mybir.dt.int16
EXP = mybir.ActivationFunctionType.Exp
MULT = mybir.AluOpType.mult
ADD = mybir.AluOpType.add

MM_DT, MM_NP = BF16, ml_dtypes.bfloat16

PV_LAG = 4
TAIL_KT = 2   # where in chunk c+1 the tail of chunk c is emitted

# two-phase Schraudolph constants: at = S(B1) + 1.16532*S(B2) where
# S(b) = bitcast_bf16(rne(A*s + 16256 + b)) ~ exp(0.125*s); max rel
# error +-1.35% (vs +-4.1% single-phase). The 1.16532 factor is folded
# into vtk_c.
A_SCH = 128.0 * 1.4426950408889634 * 0.125
B_SCH1 = 127.0 * 128.0 - 185.2977
B_SCH2 = 127.0 * 128.0 - 122.7560
C_SCH2 = 1.1653249
DVE_KT = frozenset((8,))


def _mha_kernel(tc, y, xT16, wq, wk, wv, woT, bq, bk, bv):
    with ExitStack() as ctx:
        _mha_kernel_inner(ctx, tc, y, xT16, wq, wk, wv, woT, bq, bk, bv)


def _mha_kernel_inner(ctx, tc, y, xT16, wq, wk, wv, woT, bq, bk, bv):
    nc = tc.nc
    pers = ctx.enter_context(tc.tile_pool(name="pers", bufs=1))

    qT = pers.tile([128, T], MM_DT, tag="qT")
    kT = pers.tile([128, T], MM_DT, tag="kT")
    vT = pers.tile([128, T], MM_DT, tag="vT")
    vtk = pers.tile([128, B * NKT, VW], MM_DT, tag="vtk")
    vtk_c = pers.tile([128, B * NKT, VW], MM_DT, tag="vtk_c")
    wq_sb = pers.tile([128, NI, OPC], MM_DT, tag="wq")
    wk_sb = pers.tile([128, NI, OPC], MM_DT, tag="wk")
    wv_sb = pers.tile([128, NI, OPC], MM_DT, tag="wv")
    woT_sb = pers.tile([128, D], MM_DT, tag="wo")
    bq_sb = pers.tile([128, 1], F32, tag="bq")
    bk_sb = pers.tile([128, 1], F32, tag="bk")
    bv_sb = pers.tile([128, 1], F32, tag="bv")
    ident = pers.tile([128, 128], MM_DT, tag="ident")

    # weights first on the gpsimd DMA queue (the first projection matmul
    # waits on wq); identity before woT/biases (chunk-0 V transposes).
    nc.gpsimd.dma_start(wq_sb[:, :, :], wq[:, :, :])
    nc.gpsimd.dma_start(wk_sb[:, :, :], wk[:, :, :])
    nc.gpsimd.dma_start(wv_sb[:, :, :], wv[:, :, :])
    make_identity(nc, ident)
    nc.gpsimd.dma_start(woT_sb, woT)
    nc.gpsimd.dma_start(bq_sb, bq)
    nc.gpsimd.dma_start(bk_sb, bk)
    nc.gpsimd.dma_start(bv_sb, bv)
    # constant ones/pad columns of vtk
    onepad = pers.tile([128, 2], F32, tag="onepad")
    nc.vector.memset(onepad[:, 0:1], 1.0)
    nc.vector.memset(onepad[:, 1:2], 0.0)
    onepad_b = bass.AP(
        tensor=onepad.tensor,
        offset=onepad.offset,
        ap=[onepad.ap[0], [0, B * NKT], onepad.ap[1]],
    )
    for h in range(HPC):
        nc.vector.tensor_copy(
            vtk[:, :, h * HW + DH : h * HW + DH + 2], onepad_b
        )
    # preload the exp table set while phase A runs
    warm = pers.tile([1, 1], F32, tag="warm")
    nc.scalar.activation(warm, onepad[0:1, 0:1], EXP)

    # Phase A: Q/K/V projections in o-major layout; V transposed to
    # token-major k-tiles right away.
    with (
        tc.tile_pool(name="psA", bufs=2, space="PSUM") as psA,
        tc.tile_pool(name="xin", bufs=4) as xin,
    ):
        for m in range(T // 1024):  # 4 double-chunks of 1024 tokens
            xa = xin.tile([128, 4, 1024], MM_DT, tag="xt")
            nc.sync.dma_start(xa, xT16[0, :, :, m * 1024 : (m + 1) * 1024])
            xb = xin.tile([128, 4, 1024], MM_DT, tag="xt")
            nc.sync.dma_start(xb, xT16[1, :, :, m * 1024 : (m + 1) * 1024])
            for half in range(2):
                t0 = m * 1024 + half * 512
                hsl = slice(half * 512, (half + 1) * 512)
                ps_q = psA.tile([128, TCH], F32, tag="ps_q")
                ps_k = psA.tile([128, TCH], F32, tag="ps_k")
                ps_v = psA.tile([128, TCH], F32, tag="ps_v")
                for i in range(NI):
                    blk, u = divmod(i, 4)
                    xs = (xa if blk == 0 else xb)[:, u, hsl]
                    st, sp = (i == 0), (i == NI - 1)
                    nc.tensor.matmul(ps_q, wq_sb[:, i, :], xs, start=st, stop=sp)
                    nc.tensor.matmul(ps_k, wk_sb[:, i, :], xs, start=st, stop=sp)
                    nc.tensor.matmul(ps_v, wv_sb[:, i, :], xs, start=st, stop=sp)
                sl = slice(t0, t0 + TCH)
                nc.vector.tensor_scalar_add(qT[:, sl], ps_q, bq_sb)
                nc.vector.tensor_scalar_add(kT[:, sl], ps_k, bk_sb)
                nc.vector.tensor_scalar_add(vT[:, sl], ps_v, bv_sb)
                for g in range(t0 // 128, (t0 + TCH) // 128):
                    ps_t = psA.tile([128, 128], MM_DT, tag="ps_t")
                    nc.tensor.transpose(ps_t, vT[:, g * 128 : (g + 1) * 128], ident)
                    for h in range(HPC):
                        nc.vector.tensor_copy(
                            vtk[:, g, h * HW : h * HW + DH],
                            ps_t[:, h * DH : (h + 1) * DH],
                        )
    # scaled V copy for the Schraudolph S2 term (ones column scales too,
    # keeping the denominator consistent)
    nc.vector.tensor_scalar_mul(vtk_c[:, :, :], vtk[:, :, :], C_SCH2)

    # Phase B: attention + output projection, software-pipelined (PV lags
    # scores by PV_LAG key tiles; chunk tails are emitted inside the next
    # chunk's score stretch).
    with (
        tc.tile_pool(name="psB", bufs=2, space="PSUM") as psB,
        tc.tile_pool(name="att", bufs=8) as att,
        tc.tile_pool(name="sch", bufs=2) as schp,
        tc.tile_pool(name="sm", bufs=3) as sm,
        tc.tile_pool(name="yo_p", bufs=4) as yo_p,
    ):
        pending = None

        def emit_tail(chunk):
            pvs, ctx_sb, q0 = chunk
            for h in range(HPC):
                # normalize: ctx rows for this head = pv[0:64] * recip(pv[64])
                rraw = sm.tile([1, QCH], F32, tag="rraw")
                nc.vector.tensor_copy(rraw, pvs[h][DH : DH + 1, :])
                rrow = sm.tile([1, QCH], F32, tag="rrow")
                nc.vector.reciprocal_approx_fast(rrow, rraw)
                nrm = sm.tile([DH, QCH], F32, tag="nrm")
                nc.gpsimd.partition_broadcast(nrm, rrow)
                nc.vector.tensor_mul(
                    ctx_sb[h * DH : (h + 1) * DH, :], pvs[h][0:DH, :], nrm
                )
            for t4 in range(QCH // 128):
                yo = yo_p.tile([128, D], BF16, tag="yo")
                for nch in range(D // 512):
                    ps_o = psB.tile([128, 512], F32, tag="ps_o")
                    nc.tensor.matmul(
                        ps_o,
                        ctx_sb[:, t4 * 128 : (t4 + 1) * 128],
                        woT_sb[:, nch * 512 : (nch + 1) * 512],
                        start=True,
                        stop=True,
                    )
                    nc.vector.tensor_copy(yo[:, nch * 512 : (nch + 1) * 512], ps_o)
                r0 = q0 + t4 * 128
                nc.sync.dma_start(y[r0 : r0 + 128, :], yo)

        for b in range(B):
            for qc in range(NQCH):
                q0 = b * S + qc * QCH
                ctx_sb = sm.tile([128, QCH], MM_DT, tag="ctx")
                at_tiles = []
                pvs = [
                    psB.tile([HW, QCH], F32, tag="ps_pv", name=f"pv{h}")
                    for h in range(HPC)
                ]

                def emit_pv(kt):
                    g = b * NKT + kt
                    start, stop = (kt == 0), (kt == NKT - 1)
                    for h in range(HPC):
                        if kt in DVE_KT:
                            s1, s2 = at_tiles[kt]
                            nc.tensor.matmul(
                                pvs[h], vtk[:, g, h * HW : (h + 1) * HW],
                                s1.bitcast(BF16)[:, h, :],
                                start=start, stop=False,
                            )
                            nc.tensor.matmul(
                                pvs[h], vtk_c[:, g, h * HW : (h + 1) * HW],
                                s2.bitcast(BF16)[:, h, :],
                                start=False, stop=stop,
                            )
                        else:
                            nc.tensor.matmul(
                                pvs[h], vtk[:, g, h * HW : (h + 1) * HW],
                                at_tiles[kt][:, h, :],
                                start=start, stop=stop,
                            )

                for kt in range(NKT):
                    g = b * NKT + kt
                    ps_s = psB.tile([128, 2, QCH], F32, tag="ps_s")
                    for h in range(HPC):
                        hs = slice(h * DH, (h + 1) * DH)
                        nc.tensor.matmul(
                            ps_s[:, h, :],
                            kT[hs, g * 128 : (g + 1) * 128],
                            qT[hs, q0 : q0 + QCH],
                            start=True,
                            stop=True,
                        )
                    if kt in DVE_KT:
                        w1 = schp.tile([128, 2, QCH], I16, tag="w1")
                        w2 = schp.tile([128, 2, QCH], I16, tag="w2")
                        nc.vector.tensor_scalar(w1, ps_s, A_SCH, B_SCH1, MULT, ADD)
                        nc.vector.tensor_scalar(w2, ps_s, A_SCH, B_SCH2, MULT, ADD)
                        at_tiles.append((w1, w2))
                    else:
                        at = att.tile([128, 2, QCH], MM_DT, tag="at")
                        nc.scalar.activation(at, ps_s, EXP, scale=0.125)
                        at_tiles.append(at)
                    if kt == TAIL_KT and pending is not None:
                        emit_tail(pending)
                        pending = None
                    if kt >= PV_LAG:
                        emit_pv(kt - PV_LAG)
                for kt in range(NKT - PV_LAG, NKT):
                    emit_pv(kt)
                pending = (pvs, ctx_sb, q0)
        emit_tail(pending)


_NC_CACHE = {}


def _build_nc(repeats=1):
    if repeats in _NC_CACHE:
        return _NC_CACHE[repeats]
    nc = bacc.Bacc("TRN2", target_bir_lowering=False, debug=False, num_devices=N_CORES)
    xT16 = nc.dram_tensor("xT16", [2, 128, 4, T], MM_DT, kind="ExternalInput").ap()
    wq = nc.dram_tensor("wq", [128, NI, OPC], MM_DT, kind="ExternalInput").ap()
    wk = nc.dram_tensor("wk", [128, NI, OPC], MM_DT, kind="ExternalInput").ap()
    wv = nc.dram_tensor("wv", [128, NI, OPC], MM_DT, kind="ExternalInput").ap()
    woT = nc.dram_tensor("woT", [128, D], MM_DT, kind="ExternalInput").ap()
    bq = nc.dram_tensor("bq", [128, 1], F32, kind="ExternalInput").ap()
    bk = nc.dram_tensor("bk", [128, 1], F32, kind="ExternalInput").ap()
    bv = nc.dram_tensor("bv", [128, 1], F32, kind="ExternalInput").ap()
    y = nc.dram_tensor("y", [T, D], BF16, kind="ExternalOutput").ap()
    with tile.TileContext(nc) as tc:
        for _ in range(repeats):
            _mha_kernel(tc, y, xT16, wq, wk, wv, woT, bq, bk, bv)
    nc.compile()
    _NC_CACHE[repeats] = nc
    return nc


def _prep_in_maps(inputs):
    x = np.asarray(inputs["x"], np.float32)
    Wq = np.asarray(inputs["Wq"], np.float32)
    Wk = np.asarray(inputs["Wk"], np.float32)
    Wv = np.asarray(inputs["Wv"], np.float32)
    Wo = np.asarray(inputs["Wo"], np.float32)
    bq = np.asarray(inputs["bq"], np.float32)
    bk = np.asarray(inputs["bk"], np.float32)
    bv = np.asarray(inputs["bv"], np.float32)

    xTr = np.ascontiguousarray(x.reshape(T, D).T)  # [D, T]
    # xT16 [blk, p, u, t] = xTr[(blk*4+u)*128 + p, t]
    xT16_np = np.ascontiguousarray(
        xTr.reshape(2, 4, 128, T).transpose(0, 2, 1, 3)
    ).astype(MM_NP)

    def _w_slice(W, c):
        # [128(p), NI, OPC]: [p, i, o] = W[c*OPC+o, i*128+p]
        A = np.ascontiguousarray(W[c * OPC : (c + 1) * OPC, :].T)  # [D, OPC]
        return np.ascontiguousarray(A.reshape(NI, 128, OPC).transpose(1, 0, 2)).astype(
            MM_NP
        )

    in_maps = []
    for c in range(N_CORES):
        sl = slice(c * OPC, (c + 1) * OPC)
        in_maps.append(
            {
                "xT16": xT16_np,
                "wq": _w_slice(Wq, c),
                "wk": _w_slice(Wk, c),
                "wv": _w_slice(Wv, c),
                "woT": np.ascontiguousarray(Wo[:, sl].T).astype(MM_NP),
                "bq": bq[sl].reshape(OPC, 1).copy(),
                "bk": bk[sl].reshape(OPC, 1).copy(),
                "bv": bv[sl].reshape(OPC, 1).copy(),
            }
        )
    return in_maps


def kernel(**inputs) -> np.ndarray:
    nc = _build_nc()
    in_maps = _prep_in_maps(inputs)
    res = run_bass_kernel_spmd(nc, in_maps, core_ids=list(range(N_CORES)))
    bo = np.asarray(inputs["bo"], np.float32)
    y = np.zeros((T, D), np.float64)
    for c in range(N_CORES):
        y += res.results[c]["y"].astype(np.float64)
    y = (y + bo).astype(np.float32)
    return y.reshape(B, S, D)


# revision 19
# speedup vs baseline: 1.3722x; 1.1353x over previous
"""Multi-head self-attention (B=2, S=2048, D=1024, H=16) on 8 Trainium2 cores.

Sharding: Megatron-style tensor parallelism on the head dimension.
Each core owns 2 heads (128 of the 1024 model dims):
  - Wq/Wk/Wv column-sharded: core c computes Q/K/V for dims [c*128,(c+1)*128)
  - attention for its 2 heads over both batches
  - Wo row-sharded: core c produces a partial output [4096, 1024]
  - host sums the 8 partials and adds bo.

Per-core device layouts (matmuls run as float32r = full-rate PE with
~tf32 multiply precision, fp32 accumulate):
  qT/kT: [128(out-dim), 4096(token)]  "o-major"
  v:     token-major k-tiles [128(token), 132] = 2x [head(64) | ones | pad]
         (the ones column makes the PV matmul also produce the softmax
          normalizer as output row 64; pad keeps the stationary free dim even,
          a float32r requirement)
  scores are computed transposed: sT[k, q] = (kT tile).T @ qT chunk, so the
  softmax sum reduces over the PARTITION dim -- done for free by the ones row
  in the PV matmul instead of a vector reduction. exp() needs no max
  subtraction: scores*0.125 are ~N(0,1) for this problem family, far from
  fp32 overflow.
"""

import os
import numpy as np
import ml_dtypes
from contextlib import ExitStack

import concourse.bass as bass
import concourse.tile as tile
from concourse import bacc, mybir
from concourse.bass_utils import run_bass_kernel_spmd
from concourse.masks import make_identity

B, S, D = 2, 2048, 1024
H, DH = 16, 64
T = B * S                  # 4096 tokens total
N_CORES = 8
OPC = D // N_CORES         # 128 out dims per core
HPC = H // N_CORES         # 2 heads per core
NI = D // 128              # 8 contraction chunks of 128
TCH = 512                  # projection token chunk
NTCH = T // TCH            # 8
QCH = 512                  # attention q chunk
NQCH = S // QCH            # 4 per batch
NKT = S // 128             # 16 key tiles per batch
HW = DH + 2                # 66 cols per head in the v tile (data|ones|pad)
VW = HPC * HW              # 132

F32 = mybir.dt.float32
F32R = mybir.dt.float32r
BF16 = mybir.dt.bfloat16
EXP = mybir.ActivationFunctionType.Exp

# all matmul operands bf16: full-rate PE, fast (FWL) weight loads,
# half the x DMA traffic, and the f32 PSUM accumulate keeps error well
# inside tolerance (measured rel err ~7e-3 vs the 2e-2 gate)
MM_DT, MM_NP = BF16, ml_dtypes.bfloat16
VT_DT = BF16


def _mha_kernel(tc, y, xT, wq, wk, wv, woT, bq, bk, bv):
    with ExitStack() as ctx:
        _mha_kernel_inner(ctx, tc, y, xT, wq, wk, wv, woT, bq, bk, bv)


def _mha_kernel_inner(ctx: ExitStack, tc, y, xT, wq, wk, wv, woT, bq, bk, bv):
    nc = tc.nc
    pers = ctx.enter_context(tc.tile_pool(name="pers", bufs=1))

    qT = pers.tile([128, T], MM_DT, tag="qT")
    kT = pers.tile([128, T], MM_DT, tag="kT")
    vT = pers.tile([128, T], VT_DT, tag="vT")
    vtk = pers.tile([128, B * NKT, VW], MM_DT, tag="vtk")
    wq_sb = pers.tile([128, NI, OPC], MM_DT, tag="wq")
    wk_sb = pers.tile([128, NI, OPC], MM_DT, tag="wk")
    wv_sb = pers.tile([128, NI, OPC], MM_DT, tag="wv")
    woT_sb = pers.tile([128, D], MM_DT, tag="wo")
    bq_sb = pers.tile([128, 1], F32, tag="bq")
    bk_sb = pers.tile([128, 1], F32, tag="bk")
    bv_sb = pers.tile([128, 1], F32, tag="bv")
    ident = pers.tile([128, 128], VT_DT, tag="ident")

    # identity first: it shares the gpsimd queue with the weight DMAs
    # and the chunk-0 V transposes wait on it. One DMA per weight tensor
    # (each dma_start costs ~0.6us of issue time on its queueing engine;
    # 28 per-chunk DMAs kept gpsimd busy until ~29us)
    make_identity(nc, ident)
    nc.gpsimd.dma_start(wq_sb[:, :, :], wq[:, :, :])
    nc.gpsimd.dma_start(wk_sb[:, :, :], wk[:, :, :])
    nc.gpsimd.dma_start(wv_sb[:, :, :], wv[:, :, :])
    nc.gpsimd.dma_start(woT_sb, woT)
    nc.gpsimd.dma_start(bq_sb, bq)
    nc.gpsimd.dma_start(bk_sb, bk)
    nc.gpsimd.dma_start(bv_sb, bv)
    # constant ones/pad columns of vtk (memset can't write float32r directly)
    onepad = pers.tile([128, 2], F32, tag="onepad")
    nc.vector.memset(onepad[:, 0:1], 1.0)
    nc.vector.memset(onepad[:, 1:2], 0.0)
    onepad_b = bass.AP(
        tensor=onepad.tensor,
        offset=onepad.offset,
        ap=[onepad.ap[0], [0, B * NKT], onepad.ap[1]],
    )
    for h in range(HPC):
        nc.vector.tensor_copy(
            vtk[:, :, h * HW + DH : h * HW + DH + 2], onepad_b
        )

    # Phase A: Q/K/V projections in o-major layout, then transpose V to
    # token-major k-tiles.
    with (
        tc.tile_pool(name="psA", bufs=2, space="PSUM") as psA,
        tc.tile_pool(name="xin", bufs=16) as xin,
    ):
        for t in range(NTCH):
            ps_q = psA.tile([128, TCH], F32, tag="ps_q")
            ps_k = psA.tile([128, TCH], F32, tag="ps_k")
            ps_v = psA.tile([128, TCH], F32, tag="ps_v")
            for i in range(NI):
                xt = xin.tile([128, TCH], MM_DT, tag="xt")
                nc.sync.dma_start(xt, xT[i, :, t * TCH : (t + 1) * TCH])
                st, sp = (i == 0), (i == NI - 1)
                nc.tensor.matmul(ps_q, wq_sb[:, i, :], xt, start=st, stop=sp)
                nc.tensor.matmul(ps_k, wk_sb[:, i, :], xt, start=st, stop=sp)
                nc.tensor.matmul(ps_v, wv_sb[:, i, :], xt, start=st, stop=sp)
            sl = slice(t * TCH, (t + 1) * TCH)
            nc.vector.tensor_scalar_add(qT[:, sl], ps_q, bq_sb)
            nc.vector.tensor_scalar_add(kT[:, sl], ps_k, bk_sb)
            nc.vector.tensor_scalar_add(vT[:, sl], ps_v, bv_sb)
            # transpose this chunk's V to token-major right away (keeps the
            # PE busy across the phase boundary)
            for g in range(t * TCH // 128, (t + 1) * TCH // 128):
                ps_t = psA.tile([128, 128], VT_DT, tag="ps_t")
                nc.tensor.transpose(ps_t, vT[:, g * 128 : (g + 1) * 128], ident)
                for h in range(HPC):
                    nc.vector.tensor_copy(
                        vtk[:, g, h * HW : h * HW + DH],
                        ps_t[:, h * DH : (h + 1) * DH],
                    )

    # Phase B: attention + output projection.
    with (
        tc.tile_pool(name="psB", bufs=2, space="PSUM") as psB,
        tc.tile_pool(name="att", bufs=8) as att,
        tc.tile_pool(name="sm", bufs=3) as sm,
        tc.tile_pool(name="yo_p", bufs=4) as yo_p,
    ):
        for b in range(B):
            for qc in range(NQCH):
                q0 = b * S + qc * QCH
                ctx_sb = sm.tile([128, QCH], MM_DT, tag="ctx")
                # Scores for BOTH heads interleaved: head h uses SBUF
                # partitions [h*64,(h+1)*64) so the two matmuls land on
                # different row-tiles of the 64x128-tiled PE array and stream
                # concurrently. One exp covers both heads' psum banks.
                at_tiles = []
                pvs = [
                    psB.tile([HW, QCH], F32, tag="ps_pv", name=f"pv{h}")
                    for h in range(HPC)
                ]

                def emit_pv(kt):
                    g = b * NKT + kt
                    for h in range(HPC):
                        nc.tensor.matmul(
                            pvs[h],
                            vtk[:, g, h * HW : (h + 1) * HW],
                            at_tiles[kt][:, h, :],
                            start=(kt == 0),
                            stop=(kt == NKT - 1),
                        )

                PV_LAG = 2
                for kt in range(NKT):
                    g = b * NKT + kt
                    ps_s = psB.tile([128, 2, QCH], F32, tag="ps_s")
                    for h in range(HPC):
                        hs = slice(h * DH, (h + 1) * DH)
                        nc.tensor.matmul(
                            ps_s[:, h, :],
                            kT[hs, g * 128 : (g + 1) * 128],
                            qT[hs, q0 : q0 + QCH],
                            start=True,
                            stop=True,
                        )
                    at = att.tile([128, 2, QCH], MM_DT, tag="at")
                    nc.scalar.activation(at, ps_s, EXP, scale=0.125)
                    at_tiles.append(at)
                    if kt >= PV_LAG:
                        emit_pv(kt - PV_LAG)
                for kt in range(NKT - PV_LAG, NKT):
                    emit_pv(kt)
                for h in range(HPC):
                    # normalize: ctx rows for this head = pv[0:64] * recip(pv[64])
                    rraw = sm.tile([1, QCH], F32, tag="rraw")
                    nc.vector.tensor_copy(rraw, pvs[h][DH : DH + 1, :])
                    rrow = sm.tile([1, QCH], F32, tag="rrow")
                    nc.vector.reciprocal_approx_fast(rrow, rraw)
                    nrm = sm.tile([DH, QCH], F32, tag="nrm")
                    nc.gpsimd.partition_broadcast(nrm, rrow)
                    nc.vector.tensor_mul(
                        ctx_sb[h * DH : (h + 1) * DH, :], pvs[h][0:DH, :], nrm
                    )
                for t4 in range(QCH // 128):
                    yo = yo_p.tile([128, D], F32, tag="yo")
                    for nch in range(D // 512):
                        ps_o = psB.tile([128, 512], F32, tag="ps_o")
                        nc.tensor.matmul(
                            ps_o,
                            ctx_sb[:, t4 * 128 : (t4 + 1) * 128],
                            woT_sb[:, nch * 512 : (nch + 1) * 512],
                            start=True,
                            stop=True,
                        )
                        nc.vector.tensor_copy(yo[:, nch * 512 : (nch + 1) * 512], ps_o)
                    r0 = q0 + t4 * 128
                    nc.sync.dma_start(y[r0 : r0 + 128, :], yo)


_NC_CACHE = {}


def _build_nc(repeats=1):
    if repeats in _NC_CACHE:
        return _NC_CACHE[repeats]
    nc = bacc.Bacc("TRN2", target_bir_lowering=False, debug=False, num_devices=N_CORES)
    xT = nc.dram_tensor("xT", [NI, 128, T], MM_DT, kind="ExternalInput").ap()
    wq = nc.dram_tensor("wq", [128, NI, OPC], MM_DT, kind="ExternalInput").ap()
    wk = nc.dram_tensor("wk", [128, NI, OPC], MM_DT, kind="ExternalInput").ap()
    wv = nc.dram_tensor("wv", [128, NI, OPC], MM_DT, kind="ExternalInput").ap()
    woT = nc.dram_tensor("woT", [128, D], MM_DT, kind="ExternalInput").ap()
    bq = nc.dram_tensor("bq", [128, 1], F32, kind="ExternalInput").ap()
    bk = nc.dram_tensor("bk", [128, 1], F32, kind="ExternalInput").ap()
    bv = nc.dram_tensor("bv", [128, 1], F32, kind="ExternalInput").ap()
    y = nc.dram_tensor("y", [T, D], F32, kind="ExternalOutput").ap()
    with tile.TileContext(nc) as tc:
        for _ in range(repeats):
            _mha_kernel(tc, y, xT, wq, wk, wv, woT, bq, bk, bv)
    nc.compile()
    _NC_CACHE[repeats] = nc
    return nc


def _prep_in_maps(inputs):
    x = np.asarray(inputs["x"], np.float32)
    Wq = np.asarray(inputs["Wq"], np.float32)
    Wk = np.asarray(inputs["Wk"], np.float32)
    Wv = np.asarray(inputs["Wv"], np.float32)
    Wo = np.asarray(inputs["Wo"], np.float32)
    bq = np.asarray(inputs["bq"], np.float32)
    bk = np.asarray(inputs["bk"], np.float32)
    bv = np.asarray(inputs["bv"], np.float32)

    xT_np = np.ascontiguousarray(x.reshape(T, D).T).reshape(NI, 128, T).astype(MM_NP)

    def _w_slice(W, c):
        # [128(p), NI, OPC]: [p, i, o] = W[c*OPC+o, i*128+p]
        A = np.ascontiguousarray(W[c * OPC : (c + 1) * OPC, :].T)  # [D, OPC]
        return np.ascontiguousarray(A.reshape(NI, 128, OPC).transpose(1, 0, 2)).astype(
            MM_NP
        )

    in_maps = []
    for c in range(N_CORES):
        sl = slice(c * OPC, (c + 1) * OPC)
        in_maps.append(
            {
                "xT": xT_np,
                "wq": _w_slice(Wq, c),
                "wk": _w_slice(Wk, c),
                "wv": _w_slice(Wv, c),
                "woT": np.ascontiguousarray(Wo[:, sl].T).astype(MM_NP),
                "bq": bq[sl].reshape(OPC, 1).copy(),
                "bk": bk[sl].reshape(OPC, 1).copy(),
                "bv": bv[sl].reshape(OPC, 1).copy(),
            }
        )
    return in_maps


def kernel(**inputs) -> np.ndarray:
    nc = _build_nc()
    in_maps = _prep_in_maps(inputs)
    res = run_bass_kernel_spmd(nc, in_maps, core_ids=list(range(N_CORES)))
    bo = np.asarray(inputs["bo"], np.float32)
    y = np.zeros((T, D), np.float64)
    for c in range(N_CORES):
        y += res.results[c]["y"].astype(np.float64)
    y = (y + bo).astype(np.float32)
    return y.reshape(B, S, D)

